# revision 47
# baseline (speedup 1.0000x reference)
"""Cosine-attention Trainium2 kernel (nn_CosineAttention_54082228191953).

Sharding: 8 NeuronCores, one attention head per core (tensor-parallel on H;
B=2 batches per core). Each core computes the qkv projection for its head,
cosine attention with per-head positional bias, and a partial output
projection plus the softmax denominator row; the host divides by the
denominator and sums the 8 head partials.

Shapes (hardcoded): B=2, N=2048, C=512, H=8, D=64.

Structure (512-wide i-chunks, seed-once + delta, pipelined exps):
  - pos_bias is batch-independent: each (ic, jt) PSUM score tile is seeded
    ONCE with the bias (fp8 DoubleRow passthrough from an SBUF-resident
    copy; both DR k-slots carry real bias rows -> 5 bias DMAs total, no
    zero slots), then b0's scores accumulate (k0^T q0), exp0 reads the
    tile, and b1's scores are formed by continuing accumulation on the SAME
    tile with one full-K matmul Delta = k1^T q1 - k0^T q0 (stationary
    kdelta=[-k0; +k1]), halving the bias-seed matmul work.
  - i-chunks are 512 wide: score tiles are one PSUM bank, giving a 4-deep
    tile rotation (chain latency hidden), 1-bank oa accumulators, and two
    dedicated banks (pq0/pq1) for the out-projection, which runs for chunk
    ic during chunk ic+1 with no PSUM contention.
  - exp emission is software-pipelined: exp1(jt-1) is emitted after
    exp0(jt), so the in-order ACT queue never stalls on the Delta matmul.
  - Scores are pre-scaled by A' = 128/ln2 (folded into qhat's norm scale
    and the host-packed bias). Most exp tiles run on ScalarE with
    scale=1/A'; a tunable subset runs on VectorE as a Schraudolph exp
    (int16(floor(S' + B')) bit-cast to bf16 ~= exp(S), zero-mean ~1.8
    percent element error on those key ranges) to balance the two engines.
  - Sqrt shares no activation-table set with Exp (a switch costs a 1283ns
    table load on ACT), so all h0 sqrts run as one pre-attention batch and
    all h1 sqrts as one batch inside i-chunk 0; stashes/copies use Copy,
    which lives in every set.
  - Phase A half 1 is emitted as slotted side work inside i-chunk 0's jt
    loop so the in-order engine queues never block on half-1 x DMAs; the
    half-1 q-normalizes (first needed by i-chunk 2) defer into i-chunk 1.
  - AV matmuls stream exp tiles a few steps behind; [V | 1] stationary
    gives the softmax denominator for free in row 64.
  - Input DMAs ride the SP queue ordered for earliest compute start
    (x half-0, then i-chunk-0 bias in jt-quarters interleaved with x
    half-1); output DMAs ride the otherwise-idle GpSimd queue.
  - NOTE: a PE p-state warm-up (dummy accumulating matmuls into a pq PSUM
    tag) breaks the compiled path's numerics even though CoreSim accepts
    it, and shifting the i-chunk-0 side-work slots later does too -- keep
    this emission order.
"""
import sys

sys.path.insert(0, "/opt/trn_rl_repo")

import math
import numpy as np
from contextlib import ExitStack

import ml_dtypes

from concourse import bacc
import concourse.mybir as mybir
import concourse.tile as tile
from concourse.bass_utils import run_bass_kernel_spmd

H, D, B, N, C = 8, 64, 2, 2048, 512
IC = 2            # i-chunks
ICW = N // IC     # 1024 i per chunk
JT = N // 128     # 16 j tiles
F32, F32R = mybir.dt.float32, mybir.dt.float32r
BF16 = mybir.dt.bfloat16
I16 = mybir.dt.int16
F8 = mybir.dt.float8e4
DR = mybir.MatmulPerfMode.DoubleRow
EXPF = mybir.ActivationFunctionType.Exp
SQRTF = mybir.ActivationFunctionType.Sqrt

APRIME = 128.0 / math.log(2.0)      # score pre-scale
BPRIME = 16248.875                  # Schraudolph bias (floor/trunc convert)
INV_A = 1.0 / APRIME
# jt values whose exp runs on VectorE (per batch), same for both ic
DVE_JT = {0: (3, 11), 1: (6, 14)}

TRACE = False          # set by test.py for profiling runs
LAST_RESULTS = None    # BassKernelResults of the last run
MARK = False           # analysis-only: emit phase-marker nops
MARKS = []


def _mark(nc, label):
    if MARK:
        MARKS.append((label, nc.gpsimd.engine_nop().ins.name))


def _build(t_val: float):
    """Build the single-core SPMD program (same program on all 8 cores)."""
    nc = bacc.Bacc("TRN2", target_bir_lowering=False, debug=False)

    # host-packed layouts:
    #   xh[b, p, half, cc, n] = x[b, half*ICW + n, cc*128 + p]
    #   wall[:, cc*192:cc*192+128] = w_qk chunk cc; [.., +128:+192] = w_v
    #     chunk cc; [0:64, 768:1280] = w_out head slice
    #   biasall[r, ic, jt, s, i] = A' * pos_bias[h][ic*1024+i, jt*128+s*64+r]
    #   identx[r, 0, r] = 1, identx[r, 1, 64+r] = 1 (DR bias passthrough)
    #   constsf: [0:2, 2] sqrt scales, [0:2, 4:132] recip broadcast selector
    #   pout[b, ic, p, nt, c] -> out row ic*ICW + nt*128 + p
    #   den rows are (ic, b) pairs at partition 32*(ic*2+b)
    xh_d = nc.dram_tensor("xh", [B, 128, 2, 4, ICW], BF16, kind="ExternalInput").ap()
    wall_d = nc.dram_tensor("wall", [128, 4 * 192 + C], BF16, kind="ExternalInput").ap()
    constsf_d = nc.dram_tensor("constsf", [128, 132], F32R, kind="ExternalInput").ap()
    ones2b_d = nc.dram_tensor("ones2b", [128, 2], BF16, kind="ExternalInput").ap()
    identx_d = nc.dram_tensor("identx", [64, 2, 128], F8, kind="ExternalInput").ap()
    biasall_d = nc.dram_tensor("biasall", [64, IC, JT, 2, ICW], F8,
                               kind="ExternalInput").ap()
    pout_d = nc.dram_tensor("pout", [B, IC, 128, ICW // 128, C], BF16,
                            kind="ExternalOutput").ap()
    den_d = nc.dram_tensor("den", [128, ICW], F32, kind="ExternalOutput").ap()

    with tile.TileContext(nc) as tc, ExitStack() as ctx:
        persist = ctx.enter_context(tc.tile_pool(name="persist", bufs=1))
        xtp = ctx.enter_context(tc.tile_pool(name="xtp", bufs=2))
        work = ctx.enter_context(tc.tile_pool(name="work", bufs=2))
        small = ctx.enter_context(tc.tile_pool(name="small", bufs=2))
        ptp = ctx.enter_context(tc.tile_pool(name="ptp", bufs=7))
        outp = ctx.enter_context(tc.tile_pool(name="outp", bufs=2))
        ps = ctx.enter_context(tc.tile_pool(name="ps", bufs=1, space="PSUM"))

        # ---- constants / weights (weights first: first projection gates A)
        wall = persist.tile([128, 4 * 192 + C], BF16, tag="wall")
        constsf = persist.tile([128, 132], F32R, tag="constsf")
        ones2b = persist.tile([128, 2], BF16, tag="ones2b")
        identx = persist.tile([64, 2, 128], F8, tag="identx")
        biasall = persist.tile([64, IC, JT, 2, ICW], F8, tag="biasall")
        nc.sync.dma_start(out=wall, in_=wall_d)
        wqk_cc = [wall[:, cc * 192:cc * 192 + 128] for cc in range(4)]
        wv_cc = [wall[:, cc * 192 + 128:(cc + 1) * 192] for cc in range(4)]
        wo_s = wall[0:D, 768:768 + C]
        sel2r = constsf[0:2, 4:132]
        tsc = constsf.bitcast(F32)[0:2, 2:3]
        # absorb the Sqrt act-table load while DMAs stream in; the warm's
        # input is a memset tile so the ACT queue head has zero DMA deps
        warm = persist.tile([2, 1], F32, tag="warm")
        nc.gpsimd.memset(warm, 1.0)
        nc.scalar.activation(out=warm, in_=warm, func=SQRTF)
        # NOTE: a PE p-state warm-up (dummy accumulating matmuls into a pq
        # PSUM tag) breaks the compiled path's numerics even though CoreSim
        # accepts it -- do not reintroduce.

        # persistent attention operands
        qstack = persist.tile([128, N], F32R, tag="qstack")   # [A't*q0^; A't*q1^]
        kb0 = persist.tile([64, N], F32R, tag="kb0")          # +k0^
        kdelta = persist.tile([128, N], F32R, tag="kdelta")   # [-k0^; +k1^]
        vaug = [persist.tile([128, JT, D + 1], BF16, tag=f"vaug{b}",
                             name=f"vaug{b}") for b in range(B)]
        dall = persist.tile([128, ICW], F32, tag="dall")

        # ---- phase A: projections + l2-normalize
        xts, qkraws, rtss = [], [], []
        sqss = {}
        for b in range(B):
            xts.append(xtp.tile([128, 4, N], BF16, tag="xt", name=f"xt{b}"))
            nc.gpsimd.memset(vaug[b], 1.0)
            qkraws.append(work.tile([128, N], F32, tag="qkraw", name=f"qkraw{b}"))
        # qkraw must survive into i-chunk 1 for the deferred h1 q-normalize
            rtss.append(small.tile([2, N], F32, tag=f"rt{b}", name=f"rt{b}", bufs=1))
        # x pieces land half-major so both batches' h0 chains finish early
        for half in range(2):
            for b in range(B):
                for cc in range(4):
                    nc.sync.dma_start(
                        out=xts[b][:, cc, half * ICW:(half + 1) * ICW],
                        in_=xh_d[b, :, half, cc, :])
        nc.sync.dma_start(out=biasall[:, 0], in_=biasall_d[:, 0])

        _mark(nc, "A:start")

        pjs = {}
        mi = 0

        def pj_chunk(half, b, f, tag=None):
            nonlocal mi
            ms = slice(half * 1024 + f * 512, half * 1024 + (f + 1) * 512)
            pj = ps.tile([128, 512], F32, tag=tag or f"st{mi % 4}",
                         name=f"pj{half}{b}{f}")
            for cc in range(4):
                nc.tensor.matmul(pj[:, :], wqk_cc[cc], xts[b][:, cc, ms],
                                 start=(cc == 0), stop=(cc == 3),
                                 skip_group_check=True)
            # split the stash/square pair across ACT and DVE by batch so the
            # per-chunk chain has fewer serial cross-engine hops (Copy and
            # Square live in every act-table set -> no table switches)
            sq = work.tile([128, 512], BF16, tag="sq",
                           name=f"sq{half}{b}{f}", bufs=3)
            if b == 0:
                nc.vector.tensor_copy(qkraws[b][:, ms], pj)
            else:
                nc.scalar.copy(qkraws[b][:, ms], pj)
            nc.vector.tensor_mul(sq, qkraws[b][:, ms], qkraws[b][:, ms])
            pjs[(half, b, f)] = (sq, ms)
            mi += 1

        # row norms + normalize, 512-chunk granularity. The ssq+sqrt parts
        # are batched so Sqrt<->Exp act-table switches stay rare (sqrt shares
        # no table set with exp; each switch costs a 1283ns table load).
        rts = {}

        def norm_sqrt(half, b, f, tag):
            sq, ms = pjs[(half, b, f)]
            ssq = ps.tile([2, 512], F32, tag=tag, name=f"ssq{half}{b}{f}")
            nc.tensor.matmul(ssq, ones2b, sq, start=True, stop=True,
                             skip_group_check=True)
            rt = small.tile([2, 512], F32, tag="rt", name=f"rt{half}{b}{f}",
                            bufs=4)
            nc.scalar.activation(out=rt, in_=ssq, func=SQRTF, scale=tsc[:, 0:1])
            rts[(half, b, f)] = rt

        rrs = {}

        def norm_mul(half, b, f, tag, do_q=True):
            rt = rts[(half, b, f)]
            _, ms = pjs[(half, b, f)]
            rr = small.tile([2, 512], F32R, tag="rr", name=f"rr{half}{b}{f}",
                            bufs=8)
            rrs[(half, b, f)] = rr
            with nc.allow_low_precision(reason="f32r holds full f32 bits"):
                nc.vector.reciprocal(rr, rt)
            rbc = ps.tile([128, 512], F32, tag=tag, name=f"rbc{half}{b}{f}")
            nc.tensor.matmul(rbc, sel2r, rr, start=True, stop=True,
                             skip_group_check=True)
            if b == 0:
                if do_q:
                    nc.vector.tensor_mul(qstack[0:64, ms], qkraws[0][0:64, ms],
                                         rbc[0:64, :])
                nc.vector.tensor_mul(kb0[:, ms], qkraws[0][64:128, ms],
                                     rbc[64:128, :])
                nc.gpsimd.tensor_scalar_mul(kdelta[0:64, ms], kb0[:, ms], -1.0)
            else:
                if do_q:
                    nc.vector.tensor_mul(qstack[64:128, ms],
                                         qkraws[1][0:64, ms], rbc[0:64, :])
                nc.vector.tensor_mul(kdelta[64:128, ms], qkraws[1][64:128, ms],
                                     rbc[64:128, :])

        def q_finish(half, b, f, tag):
            # deferred q-normalize: h1 q columns are first used by i-chunk 2
            rr = rrs[(half, b, f)]
            _, ms = pjs[(half, b, f)]
            rbc = ps.tile([128, 512], F32, tag=tag, name=f"rbq{half}{b}{f}")
            nc.tensor.matmul(rbc, sel2r, rr, start=True, stop=True,
                             skip_group_check=True)
            if b == 0:
                nc.vector.tensor_mul(qstack[0:64, ms], qkraws[0][0:64, ms],
                                     rbc[0:64, :])
            else:
                nc.vector.tensor_mul(qstack[64:128, ms], qkraws[1][0:64, ms],
                                     rbc[0:64, :])

        def v_group(b, g, tag):
            vt4 = ps.tile([128, 4, D], F32, tag=tag, name=f"vt4{b}{g}")
            for q in range(4):
                jt = 4 * g + q
                for cc in range(4):
                    nc.tensor.matmul(vt4[:, q, :],
                                     xts[b][:, cc, jt * 128:(jt + 1) * 128],
                                     wv_cc[cc], start=(cc == 0), stop=(cc == 3),
                                     skip_group_check=True)
            nc.vector.tensor_copy(vaug[b][:, 4 * g:4 * g + 4, 0:D], vt4)

        # half-0 projection chunks + their sqrts, then the exp table warm:
        # everything else rides inside i-chunk 0's loop as side work. All
        # sqrts stay contiguous on ACT (sqrt shares no act-table set with
        # exp; Copy/stash does, so only sqrt batches force table loads).
        for f in range(2):
            for b in range(B):
                pj_chunk(0, b, f)
        for f in range(2):
            for b in range(B):
                norm_sqrt(0, b, f, "oa0")
        nc.scalar.activation(out=warm, in_=tsc, func=EXPF)
        for f in range(2):
            for b in range(B):
                norm_mul(0, b, f, "oa1")
        _mark(nc, "A:h0done")

        def sqrt_batch():
            for b in range(B):
                for f in range(2):
                    norm_sqrt(1, b, f, f"pq{f}")

        # jt -> side work for i-chunk 0: half-1 of phase A, scheduled to land
        # just after its x DMAs and just before its attention consumers
        side_h1 = {
            1: [lambda: v_group(0, 0, "pq0"), lambda: v_group(1, 0, "pq1")],
            2: [lambda: v_group(0, 1, "pq0"), lambda: v_group(1, 1, "pq1")],
            3: [lambda: pj_chunk(1, 0, 0, "pq0")],
            4: [lambda: pj_chunk(1, 0, 1, "pq1")],
            5: [lambda: pj_chunk(1, 1, 0, "pq0"),
                lambda: pj_chunk(1, 1, 1, "pq1")],
            6: [sqrt_batch],
            7: [lambda: norm_mul(1, 0, 0, "pq0", do_q=False),
                lambda: norm_mul(1, 1, 0, "pq1", do_q=False)],
            8: [lambda: norm_mul(1, 0, 1, "pq0", do_q=False),
                lambda: norm_mul(1, 1, 1, "pq1", do_q=False)],
            9: [lambda: v_group(0, 2, "pq0")],
            10: [lambda: v_group(1, 2, "pq1")],
            11: [lambda: v_group(0, 3, "pq0")],
            12: [lambda: v_group(1, 3, "pq1")],
        }

        # ---- phase B: attention (seed-once + delta), out-proj overlapped
        def emit_exp(jt, b, st, pt):
            if jt in DVE_JT[b]:
                nc.vector.tensor_scalar_add(pt.bitcast(I16), st, BPRIME)
            else:
                nc.scalar.activation(out=pt, in_=st, func=EXPF, scale=INV_A)

        def flush_av(oa, item):
            b2, jt2, pt2 = item
            for f in range(2):
                fs = slice(f * 512, (f + 1) * 512)
                nc.tensor.matmul(oa[b2][:, fs], vaug[b2][:, jt2, :], pt2[:, fs],
                                 start=(jt2 == 0), stop=(jt2 == JT - 1),
                                 skip_group_check=True)

        ots = {}

        def outproj_chunk(b, ic, attnT, nt, use_act=False,
                          tags=("pq0", "pq1")):
            if nt == 0:
                ots[(b, ic)] = outp.tile([128, ICW // 128, C], BF16, tag="ot",
                                         name=f"ot{b}{ic}")
            ot = ots[(b, ic)]
            pq = ps.tile([128, C], F32, tag=tags[nt % len(tags)], name="pq")
            nc.tensor.matmul(pq, attnT[:, nt * 128:(nt + 1) * 128], wo_s,
                             start=True, stop=True, skip_group_check=True)
            if use_act and nt % 2 == 1:
                nc.scalar.copy(ot[:, nt, :], pq)
            else:
                nc.vector.tensor_copy(ot[:, nt, :], pq)
            if nt == ICW // 128 - 1:
                nc.gpsimd.dma_start(out=pout_d[b, ic], in_=ot)

        post = []  # (b, ic, attnT)
        for ic in range(IC):
            _mark(nc, f"B:ic{ic}")
            i0 = ic * ICW
            if ic == 0:
                side = dict(side_h1)
            else:
                side = {}
                slot = 0
                for b2, ic2, attnT2 in post:
                    for nt in range(ICW // 128):
                        side[slot] = [
                            lambda b2=b2, ic2=ic2, attnT2=attnT2, nt=nt:
                            outproj_chunk(b2, ic2, attnT2, nt)]
                        slot += 1
                post = []
                if ic == 1:
                    side[8] = [lambda: q_finish(1, 0, 0, "pq0"),
                               lambda: q_finish(1, 1, 0, "pq1")]
                    side[9] = [lambda: q_finish(1, 0, 1, "pq0"),
                               lambda: q_finish(1, 1, 1, "pq1")]
            oa = [ps.tile([D + 1, ICW], F32, tag=f"oa{b}", name=f"oa{ic}{b}")
                  for b in range(B)]
            pend = []
            prev = None  # (jt, st) awaiting Delta + exp1
            for jt in range(JT):
                if jt == 8:
                    _mark(nc, f"B:ic{ic}:jt8")
                st = ps.tile([128, ICW], F32, tag=f"st{jt % 4}",
                             name=f"st{ic}_{jt}")
                nc.tensor.matmul(st, identx, biasall[:, ic, jt, :, :],
                                 start=True, stop=False, perf_mode=DR,
                                 skip_group_check=True)
                nc.tensor.matmul(st, kb0[:, jt * 128:(jt + 1) * 128],
                                 qstack[0:64, i0:i0 + ICW],
                                 start=False, stop=True, skip_group_check=True)
                pt0 = ptp.tile([128, ICW], BF16, tag="pt", name=f"pt{ic}_{jt}_0")
                emit_exp(ic, jt, 0, st, pt0)
                pend.append((0, jt, pt0))
                if prev is not None:
                    jt1, st1 = prev
                    nc.tensor.matmul(st1, kdelta[:, jt1 * 128:(jt1 + 1) * 128],
                                     qstack[:, i0:i0 + ICW],
                                     start=False, stop=True,
                                     skip_group_check=True)
                    pt1 = ptp.tile([128, ICW], BF16, tag="pt",
                                   name=f"pt{ic}_{jt1}_1")
                    emit_exp(ic, jt1, 1, st1, pt1)
                    pend.append((1, jt1, pt1))
                prev = (jt, st)
                for w in side.pop(jt, ()):
                    w()
                while len(pend) > 5:
                    flush_av(oa, pend.pop(0))
            jt1, st1 = prev
            nc.tensor.matmul(st1, kdelta[:, jt1 * 128:(jt1 + 1) * 128],
                             qstack[:, i0:i0 + ICW],
                             start=False, stop=True, skip_group_check=True)
            pt1 = ptp.tile([128, ICW], BF16, tag="pt", name=f"pt{ic}_{jt1}_1")
            emit_exp(ic, jt1, 1, st1, pt1)
            pend.append((1, jt1, pt1))
            for ws in [side[k] for k in sorted(side)]:
                for w in ws:
                    w()
            side = {}
            _mark(nc, f"B:ic{ic}:done")

            def finish_b(b):
                attnT = small.tile([D, ICW], BF16, tag=f"attnT{b}",
                                   name=f"attnT{ic}{b}", bufs=2)
                if b == 0:
                    nc.vector.tensor_copy(attnT, oa[b][0:D, :])
                else:
                    nc.scalar.copy(attnT, oa[b][0:D, :])
                kk = 32 * (2 * (ic % 2) + b)
                nc.vector.tensor_copy(dall[kk:kk + 1, :], oa[b][D:D + 1, :])
                nc.gpsimd.dma_start(out=den_d[ic, b], in_=dall[kk:kk + 1, :])
                post.append((b, ic, attnT))

            # drain batch 0 first so its attnT copy overlaps batch 1's AVs
            for item in [p for p in pend if p[0] == 0]:
                flush_av(oa, item)
            finish_b(0)
            if ic == IC - 1:
                for nt in range(ICW // 128):
                    outproj_chunk(0, ic, post[-1][2], nt, use_act=True,
                                  tags=("st0", "st1", "pq0", "pq1"))
            for item in [p for p in pend if p[0] == 1]:
                flush_av(oa, item)
            pend = []
            finish_b(1)
            if ic == IC - 1:
                for nt in range(ICW // 128):
                    outproj_chunk(1, ic, post[-1][2], nt, use_act=True,
                                  tags=("st2", "st3", "pq0", "pq1"))
                post = []

        # tail: last chunk's out-projection (all four PSUM tags free)
        _mark(nc, "OP:tail")
        for k, (b2, ic2, attnT2) in enumerate(post):
            emit_outproj(b2, ic2, attnT2, ["st0", "st1", "oa0", "oa1"], k * 4)

    nc.compile()
    return nc


def _run_device(x, w_qkv, w_out, pos_bias, t_val):
    global LAST_RESULTS
    nc = _build(t_val)

    x = np.asarray(x, dtype=np.float32)
    w_qkv = np.asarray(w_qkv, dtype=np.float32)
    w_out = np.asarray(w_out, dtype=np.float32)
    pos_bias = np.asarray(pos_bias, dtype=np.float32)

    bf16 = ml_dtypes.bfloat16
    fp8 = ml_dtypes.float8_e4m3
    constsf = np.zeros((128, 132), dtype=np.float32)
    constsf[0, 2] = 1.0 / (t_val * APRIME) ** 2  # sqrt scale, q row
    constsf[1, 2] = 1.0                          # sqrt scale, k row
    constsf[0, 4:68] = 1.0       # sel2 row 0 -> partitions 0-63
    constsf[1, 68:132] = 1.0     # sel2 row 1 -> partitions 64-127
    ones2b = np.zeros((128, 2), dtype=np.float32)
    ones2b[0:64, 0] = 1.0
    ones2b[64:128, 1] = 1.0
    identx = np.zeros((64, 2, 128), dtype=np.float32)
    for r in range(64):
        identx[r, 0, r] = 1.0
        identx[r, 1, 64 + r] = 1.0
    # xh[b, p, half, cc, i] = x[b, half*ICW + i, cc*128 + p]
    xT = x.transpose(0, 2, 1)                                 # [B, C, N]
    xT = xT.reshape(B, 4, 128, 2, ICW)                        # [B, cc, p, half, i]
    xh = np.ascontiguousarray(xT.transpose(0, 2, 3, 1, 4)).astype(bf16)
    w3 = w_qkv.reshape(C, H, D, 3)
    in_maps = []
    for h in range(H):
        wall = np.zeros((128, 4 * 192 + C), dtype=np.float32)
        for cc in range(4):
            rows = slice(cc * 128, (cc + 1) * 128)
            wall[:, cc * 192:cc * 192 + D] = w3[rows, h, :, 0]
            wall[:, cc * 192 + D:cc * 192 + 128] = w3[rows, h, :, 1]
            wall[:, cc * 192 + 128:(cc + 1) * 192] = w3[rows, h, :, 2]
        wall[0:D, 768:768 + C] = w_out[h * D:(h + 1) * D, :]
        # biasall[r, ic, jt, s, i] = A' * pos_bias[h].T[jt*128+s*64+r, ic*1024+i]
        bT = (APRIME * pos_bias[h].T).reshape(JT, 2, 64, IC, ICW)
        biasall = np.ascontiguousarray(bT.transpose(2, 3, 0, 1, 4)).astype(fp8)
        in_maps.append({
            "xh": xh,
            "wall": wall.astype(bf16),
            "constsf": constsf,
            "ones2b": ones2b.astype(bf16),
            "identx": identx.astype(fp8),
            "biasall": biasall,
        })

    res = run_bass_kernel_spmd(nc, in_maps, list(range(H)), trace=TRACE)
    LAST_RESULTS = res
    acc = np.zeros((B, N, C), dtype=np.float64)
    for h in range(H):
        # pout[b, ic, p, nt, c] -> row ic*ICW + nt*128 + p; den k = ic*2 + b
        pout = np.asarray(res.results[h]["pout"], dtype=np.float64)
        pout = pout.transpose(0, 1, 3, 2, 4).reshape(B, N, C)
        den = np.asarray(res.results[h]["den"], dtype=np.float64)
        den = den[[0, 32, 64, 96]].reshape(IC, B, ICW).transpose(1, 0, 2).reshape(B, N)
        acc += pout / den[:, :, None]
    return acc.astype(np.float32)


def _reference_numpy(x, w_qkv, w_out, pos_bias, temperature, mask):
    """Exact-math fallback (used only when mask has padded positions)."""
    x = np.asarray(x, dtype=np.float32)
    qkv = (x @ np.asarray(w_qkv)).reshape(B, N, H, D, 3)
    qkv = np.transpose(qkv, (4, 0, 2, 1, 3))
    q, k, v = qkv[0], qkv[1], qkv[2]

    def l2n(t):
        n = np.linalg.norm(t, axis=-1, keepdims=True)
        return t / np.maximum(n, 1e-12)

    q, k = l2n(q), l2n(k)
    dots = np.einsum("bhid,bhjd->bhij", q, k) * np.float32(temperature)
    dots = dots + np.asarray(pos_bias)[None]
    valid = ~np.asarray(mask)
    am = ~(valid[:, None, :, None] & valid[:, None, None, :])
    dots = np.where(am, -np.finfo(np.float32).max, dots)
    dots = dots - dots.max(axis=-1, keepdims=True)
    e = np.exp(dots)
    attn = e / e.sum(axis=-1, keepdims=True)
    out = np.einsum("bhij,bhjd->bhid", attn, v)
    out = np.transpose(out, (0, 2, 1, 3)).reshape(B, N, H * D)
    return (out @ np.asarray(w_out)).astype(np.float32)


def kernel(x, w_qkv, w_out, pos_bias, temperature, mask):
    mask = np.asarray(mask)
    t_val = float(np.asarray(temperature))
    if mask.any():
        return _reference_numpy(x, w_qkv, w_out, pos_bias, t_val, mask)
    return _run_device(x, w_qkv, w_out, pos_bias, t_val)
